# revision 10
# baseline (speedup 1.0000x reference)
"""AI4DEM 5x5x5-stencil DEM single step on 8 TRN2 NeuronCores.

Sharding: Z (dim 2, 256 planes) split 8 ways -> 32 planes/core. Halos (stencil
radius 2, circular per jnp.roll semantics) are materialized host-side by
wrap-padding, so cores are fully independent for the single step.

Device layout per core: partition axis = Y (128). Per z-chunk of DZ=8 output
planes, each field is loaded as [128, (DZ+4)*136] fp32 tiles (z planes
contiguous in the free dim, x padded by 4 per side) in 5 y-rolled variants so
every one of the 124 nonzero (sz,sy,sx) stencil shifts is a pure
free-dim-offset AP. Contact+damping force accumulators (they only ever appear
summed) are accumulated in PSUM via identity matmuls on the TensorEngine.
"""

import os
import sys

sys.path.insert(0, "/opt/trn_rl_repo")

import numpy as np

N_CORES = 8
Z, Y, X = 256, 128, 128
ZC = Z // N_CORES          # z planes per core
DZ = 8                     # z planes per chunk
NCHUNK = int(os.environ.get("DEM_NCHUNK", ZC // DZ))
HW_ROW = 136               # x row width with 4+4 halo
PLANES = DZ + 4            # z planes resident per chunk
TFREE = PLANES * HW_ROW    # free size of an input tile
WIN = 7 * HW_ROW + 128     # flat compute window (plane2 x0 .. plane9 x127)
W0 = 2 * HW_ROW + 4        # window start within a base tile
FDOUT = DZ * X             # 1024, psum accumulator / epilogue free size

CELL = 0.003
D = CELL
TWO_D = 2.0 * D
KN = 10000.0
_REST = 0.5
_ALPHA = -np.log(_REST) / np.pi
_GAMMA = _ALPHA / np.sqrt(_ALPHA**2 + 1.0)
RHO_P = 1592.0
MASS = 4.0 / 3.0 * 3.1415 * CELL**3 * RHO_P
ETA = 2.0 * _GAMMA * np.sqrt(KN * MASS / 2.0)
ETA_WALL = 2.0 * _GAMMA * np.sqrt(KN * MASS)
DT = 0.0001
EPS = 0.0001
LX, LY, LZ = X * CELL, Y * CELL, Z * CELL
C_F = DT / MASS            # velocity update coefficient
R_ED = ETA / KN
HIT_THR = TWO_D * TWO_D          # hit test on q = d2

_SHIFTS = [
    (sz, sy, sx)
    for sy in (0, -1, 1, -2, 2)
    for sz in (-2, -1, 0, 1, 2)
    for sx in (-2, -1, 0, 1, 2)
    if not (sz == 0 and sy == 0 and sx == 0)
]


def build_nc():
    from concourse import bacc, mybir, masks
    from concourse.tile import TileContext

    f32 = mybir.dt.float32
    A = mybir.AluOpType

    nc = bacc.Bacc()
    pad = nc.declare_dram_parameter(
        "pad", [6, ZC + 4, Y + 4, HW_ROW], f32, isOutput=False
    )
    msk = nc.declare_dram_parameter("msk", [ZC, Y, X], f32, isOutput=False)
    out = nc.declare_dram_parameter("out", [6, ZC, Y, X], f32, isOutput=True)

    with TileContext(nc) as tc:
        with (
            tc.tile_pool(name="const", bufs=1) as constp,
            tc.tile_pool(name="base", bufs=1) as basep,
            tc.tile_pool(name="roll", bufs=1) as rollp,
            tc.tile_pool(name="long", bufs=2) as longp,
            tc.tile_pool(name="scr", bufs=12) as scrp,
            tc.tile_pool(name="psum", bufs=1, space="PSUM") as psump,
        ):
            ident = constp.tile([128, 128], f32, tag="ident")
            masks.make_identity(nc, ident[:, :])

            for ck in range(NCHUNK):
                z0 = ck * DZ

                # ---- load base (sy=0) tiles + mask ----
                base = {}
                for f in range(6):
                    t = basep.tile([128, TFREE], f32, tag=f"base{f}")
                    t3 = t[:, :].rearrange("p (z x) -> p z x", z=PLANES)
                    nc.sync.dma_start(
                        out=t3,
                        in_=pad[f, z0 : z0 + PLANES, 2 : 2 + 128, :].transpose(
                            [1, 0, 2]
                        ),
                    )
                    base[f] = t
                mt = constp.tile([128, FDOUT], f32, tag="mask")
                nc.sync.dma_start(
                    out=mt[:, :].rearrange("p (z x) -> p z x", z=DZ),
                    in_=msk[z0 : z0 + DZ, :, :].transpose([1, 0, 2]),
                )

                # ---- PSUM accumulators ----
                wacc = [
                    psump.tile([128, FDOUT], f32, tag=f"w{a}", name=f"wacc{a}")
                    for a in range(3)
                ]

                first = True
                for sy in (0, -1, 1, -2, 2):
                    if sy == 0:
                        cur = base
                    else:
                        cur = {}
                        for f in range(6):
                            t = rollp.tile([128, TFREE], f32, tag=f"roll{f}")
                            t3 = t[:, :].rearrange("p (z x) -> p z x", z=PLANES)
                            nc.sync.dma_start(
                                out=t3,
                                in_=pad[
                                    f,
                                    z0 : z0 + PLANES,
                                    2 - sy : 130 - sy,
                                    :,
                                ].transpose([1, 0, 2]),
                            )
                            cur[f] = t
                    for sz in (-2, -1, 0, 1, 2):
                        for sx in (-2, -1, 0, 1, 2):
                            if sz == 0 and sy == 0 and sx == 0:
                                continue
                            no = W0 - sz * HW_ROW - sx  # neighbor window start
                            cw = [base[f][:, W0 : W0 + WIN] for f in range(6)]
                            nw = [cur[f][:, no : no + WIN] for f in range(6)]

                            dx = longp.tile([128, WIN], f32, tag="dx")
                            dy = longp.tile([128, WIN], f32, tag="dy")
                            dzt = longp.tile([128, WIN], f32, tag="dz")
                            nc.vector.tensor_tensor(dx[:, :], cw[0], nw[0], A.subtract)
                            nc.vector.tensor_tensor(dy[:, :], cw[1], nw[1], A.subtract)
                            nc.vector.tensor_tensor(dzt[:, :], cw[2], nw[2], A.subtract)

                            x2 = scrp.tile([128, WIN], f32, tag="scr")
                            y2 = scrp.tile([128, WIN], f32, tag="scr")
                            z2 = scrp.tile([128, WIN], f32, tag="scr")
                            nc.scalar.square(x2[:, :], dx[:, :])
                            nc.scalar.square(y2[:, :], dy[:, :])
                            nc.scalar.square(z2[:, :], dzt[:, :])
                            s12 = scrp.tile([128, WIN], f32, tag="scr")
                            nc.vector.tensor_tensor(s12[:, :], x2[:, :], y2[:, :], A.add)
                            q = longp.tile([128, WIN], f32, tag="q")
                            nc.vector.tensor_tensor(q[:, :], z2[:, :], s12[:, :], A.add)

                            dvx = scrp.tile([128, WIN], f32, tag="scr")
                            dvy = scrp.tile([128, WIN], f32, tag="scr")
                            dvz = scrp.tile([128, WIN], f32, tag="scr")
                            nc.vector.tensor_tensor(dvx[:, :], cw[3], nw[3], A.subtract)
                            nc.vector.tensor_tensor(dvy[:, :], cw[4], nw[4], A.subtract)
                            nc.vector.tensor_tensor(dvz[:, :], cw[5], nw[5], A.subtract)
                            m1 = scrp.tile([128, WIN], f32, tag="scr")
                            m2 = scrp.tile([128, WIN], f32, tag="scr")
                            nc.vector.tensor_tensor(m1[:, :], dvx[:, :], dx[:, :], A.mult)
                            nc.vector.tensor_tensor(m2[:, :], dvy[:, :], dy[:, :], A.mult)
                            # dist = sqrt(q) on ScalarE; denom = max(dist, EPS);
                            # inv = 1/denom (fast NR reciprocal, ~18 bits)
                            dist = scrp.tile([128, WIN], f32, tag="scr")
                            nc.scalar.sqrt(dist[:, :], q[:, :])
                            den = scrp.tile([128, WIN], f32, tag="scr")
                            nc.vector.tensor_scalar(
                                den[:, :], dist[:, :], EPS, None, A.max
                            )
                            inv = longp.tile([128, WIN], f32, tag="inv")
                            nc.vector.reciprocal_approx_fast(inv[:, :], den[:, :])

                            # F = ((dist - 2d) + R_ED*dvn*inv) * inv
                            a1 = scrp.tile([128, WIN], f32, tag="scr")
                            nc.vector.tensor_scalar(
                                a1[:, :], dist[:, :], TWO_D, None, A.subtract
                            )
                            b1 = scrp.tile([128, WIN], f32, tag="scr")
                            nc.vector.tensor_tensor(b1[:, :], m1[:, :], inv[:, :], A.mult)
                            c1 = scrp.tile([128, WIN], f32, tag="scr")
                            nc.vector.scalar_tensor_tensor(
                                c1[:, :], b1[:, :], R_ED, a1[:, :], A.mult, A.add
                            )
                            F = scrp.tile([128, WIN], f32, tag="scr")
                            nc.vector.tensor_tensor(F[:, :], c1[:, :], inv[:, :], A.mult)
                            hit = scrp.tile([128, WIN], f32, tag="scr")
                            nc.vector.tensor_scalar(
                                hit[:, :], q[:, :], HIT_THR, None, A.is_lt
                            )
                            Wt = longp.tile([128, WIN], f32, tag="W")
                            nc.vector.tensor_tensor(Wt[:, :], F[:, :], hit[:, :], A.mult)

                            last = (sy == 2) and (sz == 2) and (sx == 2)
                            for a, dd in enumerate((dx, dy, dzt)):
                                P = scrp.tile([128, 8 * HW_ROW], f32, tag="scr")
                                nc.vector.tensor_tensor(
                                    P[:, 0:WIN], Wt[:, :], dd[:, :], A.mult
                                )
                                for g in range(2):  # two 512-wide psum groups
                                    rhs = (
                                        P[:, g * 544 : g * 544 + 544]
                                        .rearrange("p (z x) -> p z x", z=4)[:, :, 0:128]
                                    )
                                    o = (
                                        wacc[a][:, g * 512 : (g + 1) * 512]
                                        .rearrange("p (z x) -> p z x", z=4)
                                    )
                                    nc.tensor.matmul(
                                        o, ident[:, :], rhs, start=first, stop=last
                                    )
                            first = False

                # ---- epilogue: walls + integration ----
                c3 = [
                    base[f][:, :]
                    .rearrange("p (z x) -> p z x", z=PLANES)[:, 2 : 2 + DZ, 4 : 4 + 128]
                    for f in range(6)
                ]
                m3 = mt[:, :].rearrange("p (z x) -> p z x", z=DZ)
                wall_cfg = [
                    # (pos_idx, vel_idx, lo_thr, hi_thr, lo_has_ne0, lo_coeff_base)
                    (0, 3, D, LX - TWO_D, True, D),
                    (1, 4, D, LY - TWO_D, True, D),
                    (2, 5, 3.0 * D, LZ - TWO_D, True, 3.0 * D),
                ]
                for a, (pi, vi, lo_thr, hi_thr, lo_ne0, lo_base) in enumerate(wall_cfg):
                    Xc, Vc = c3[pi], c3[vi]
                    wv = wacc[a][:, :].rearrange("p (z x) -> p z x", z=DZ)
                    il = scrp.tile([128, FDOUT], f32, tag="scr")
                    il3 = il[:, :].rearrange("p (z x) -> p z x", z=DZ)
                    t_a = scrp.tile([128, FDOUT], f32, tag="scr")
                    t_a3 = t_a[:, :].rearrange("p (z x) -> p z x", z=DZ)
                    # il = (pos < lo_thr) & (pos != 0)
                    nc.vector.tensor_scalar(il3, Xc, lo_thr, None, A.is_lt)
                    nc.vector.tensor_scalar(t_a3, Xc, 0.0, None, A.not_equal)
                    nc.vector.tensor_tensor(il3, il3, t_a3, A.mult)
                    ir = scrp.tile([128, FDOUT], f32, tag="scr")
                    ir3 = ir[:, :].rearrange("p (z x) -> p z x", z=DZ)
                    nc.vector.tensor_scalar(ir3, Xc, hi_thr, None, A.is_gt)
                    # wall spring: il*(lo_base - pos) - ir*(pos - hi_thr)
                    u1 = scrp.tile([128, FDOUT], f32, tag="scr")
                    u13 = u1[:, :].rearrange("p (z x) -> p z x", z=DZ)
                    nc.vector.tensor_scalar(u13, Xc, lo_base, -1.0, A.subtract, A.mult)
                    nc.vector.tensor_tensor(u13, u13, il3, A.mult)
                    u2 = scrp.tile([128, FDOUT], f32, tag="scr")
                    u23 = u2[:, :].rearrange("p (z x) -> p z x", z=DZ)
                    nc.vector.tensor_scalar(u23, Xc, hi_thr, None, A.subtract)
                    nc.vector.tensor_tensor(u23, u23, ir3, A.mult)
                    nc.vector.tensor_tensor(u13, u13, u23, A.subtract)
                    # g1 = wall - wacc  (all forces / KN)
                    nc.vector.tensor_tensor(u13, u13, wv, A.subtract)
                    # damp indicator sum
                    nc.vector.tensor_tensor(ir3, il3, ir3, A.add)
                    # g2 = (-C_F*ETA_WALL * vel) * (il+ir)
                    nc.vector.scalar_tensor_tensor(
                        ir3, Vc, -C_F * ETA_WALL, ir3, A.mult, A.mult
                    )
                    # g3 = C_F*KN*g1 + g2
                    nc.vector.scalar_tensor_tensor(
                        u13, u13, C_F * KN, ir3, A.mult, A.add
                    )
                    if a == 2:
                        nc.vector.tensor_scalar(u13, u13, DT * -9.8, None, A.add)
                    # masked
                    nc.vector.tensor_tensor(u13, u13, m3, A.mult)
                    vn = scrp.tile([128, FDOUT], f32, tag="scr")
                    vn3 = vn[:, :].rearrange("p (z x) -> p z x", z=DZ)
                    nc.vector.tensor_tensor(vn3, Vc, u13, A.add)
                    xn = scrp.tile([128, FDOUT], f32, tag="scr")
                    xn3 = xn[:, :].rearrange("p (z x) -> p z x", z=DZ)
                    nc.vector.scalar_tensor_tensor(xn3, vn3, DT, Xc, A.mult, A.add)
                    nc.sync.dma_start(
                        out=out[a, z0 : z0 + DZ, :, :].transpose([1, 0, 2]), in_=xn3
                    )
                    nc.sync.dma_start(
                        out=out[3 + a, z0 : z0 + DZ, :, :].transpose([1, 0, 2]),
                        in_=vn3,
                    )
    nc.compile()
    return nc


_NC = None


def _get_nc():
    global _NC
    if _NC is None:
        _NC = build_nc()
    return _NC


def shard_inputs(x_grid, y_grid, z_grid, vx_grid, vy_grid, vz_grid, mask):
    F = np.stack(
        [
            np.asarray(a, dtype=np.float32).reshape(Z, Y, X)
            for a in (x_grid, y_grid, z_grid, vx_grid, vy_grid, vz_grid)
        ]
    )
    Fp = np.pad(F, ((0, 0), (2, 2), (2, 2), (4, 4)), mode="wrap")
    mk = np.asarray(mask, dtype=np.float32).reshape(Z, Y, X)
    in_maps = []
    for c in range(N_CORES):
        in_maps.append(
            {
                "pad": np.ascontiguousarray(Fp[:, c * ZC : c * ZC + ZC + 4]),
                "msk": np.ascontiguousarray(mk[c * ZC : c * ZC + ZC]),
            }
        )
    return in_maps


def assemble(results):
    full = np.empty((6, 1, 1, Z, Y, X), dtype=np.float32)
    for c in range(N_CORES):
        full[:, 0, 0, c * ZC : (c + 1) * ZC] = results[c]["out"]
    return full


def kernel(**inputs):
    from concourse.bass_utils import run_bass_kernel_spmd

    nc = _get_nc()
    in_maps = shard_inputs(**inputs)
    res = run_bass_kernel_spmd(nc, in_maps, list(range(N_CORES)))
    return assemble(res.results)


# revision 15
# speedup vs baseline: 1.5230x; 1.5230x over previous
"""AI4DEM 5x5x5-stencil DEM single step on 8 TRN2 NeuronCores.

Sharding: Z (dim 2, 256 planes) split 8 ways -> 32 planes/core. Halos (stencil
radius 2, circular per jnp.roll semantics) are materialized host-side by
wrap-padding, so cores are fully independent for the single step.

Device layout per core: partition axis = Y (128). Per z-chunk of DZ=8 output
planes, each field is loaded as [128, (DZ+4)*136] fp32 tiles (z planes
contiguous in the free dim, x padded by 4 per side) in 5 y-rolled variants so
every one of the 124 nonzero (sz,sy,sx) stencil shifts is a pure
free-dim-offset AP. Contact+damping force accumulators (they only ever appear
summed) are accumulated in PSUM via identity matmuls on the TensorEngine.
"""

import os
import sys

sys.path.insert(0, "/opt/trn_rl_repo")

import numpy as np

N_CORES = 8
Z, Y, X = 256, 128, 128
ZC = Z // N_CORES          # z planes per core
DZ = 8                     # z planes per chunk
NCHUNK = int(os.environ.get("DEM_NCHUNK", ZC // DZ))
HW_ROW = 136               # x row width with 4+4 halo
PLANES = DZ + 4            # z planes resident per chunk
TFREE = PLANES * HW_ROW    # free size of an input tile
WIN = 7 * HW_ROW + 128     # flat compute window (plane2 x0 .. plane9 x127)
W0 = 2 * HW_ROW + 4        # window start within a base tile
FDOUT = DZ * X             # 1024, psum accumulator / epilogue free size

CELL = 0.003
D = CELL
TWO_D = 2.0 * D
KN = 10000.0
_REST = 0.5
_ALPHA = -np.log(_REST) / np.pi
_GAMMA = _ALPHA / np.sqrt(_ALPHA**2 + 1.0)
RHO_P = 1592.0
MASS = 4.0 / 3.0 * 3.1415 * CELL**3 * RHO_P
ETA = 2.0 * _GAMMA * np.sqrt(KN * MASS / 2.0)
ETA_WALL = 2.0 * _GAMMA * np.sqrt(KN * MASS)
DT = 0.0001
EPS = 0.0001
LX, LY, LZ = X * CELL, Y * CELL, Z * CELL
C_F = DT / MASS            # velocity update coefficient
R_ED = ETA / KN
HIT_THR = TWO_D * TWO_D          # hit test on q = d2

_SHIFTS = [
    (sz, sy, sx)
    for sy in (0, -1, 1, -2, 2)
    for sz in (-2, -1, 0, 1, 2)
    for sx in (-2, -1, 0, 1, 2)
    if not (sz == 0 and sy == 0 and sx == 0)
]


def build_nc():
    from concourse import bacc, mybir, masks
    from concourse.tile import TileContext

    f32 = mybir.dt.float32
    bf16 = mybir.dt.bfloat16
    A = mybir.AluOpType

    nc = bacc.Bacc()
    # const AP for the ScalarE activation bias (dist - 2d)
    _cb = nc.alloc_sbuf_tensor("const-f32-m2d", [128, 1], f32)
    nc.gpsimd.memset(_cb.ap(), -TWO_D)
    nc.const_aps.aps[(f32, -TWO_D)] = _cb.ap()
    nc.all_engine_barrier()
    pad = nc.declare_dram_parameter(
        "pad", [6, ZC + 4, Y + 4, HW_ROW], f32, isOutput=False
    )
    msk = nc.declare_dram_parameter("msk", [ZC, Y, X], f32, isOutput=False)
    out = nc.declare_dram_parameter("out", [6, ZC, Y, X], f32, isOutput=True)

    with TileContext(nc) as tc:
        with (
            tc.tile_pool(name="const", bufs=1) as constp,
            tc.tile_pool(name="base", bufs=1) as basep,
            tc.tile_pool(name="roll", bufs=1) as rollp,
            tc.tile_pool(name="long", bufs=2) as longp,
            tc.tile_pool(name="scr", bufs=12) as scrp,
            tc.tile_pool(name="psum", bufs=1, space="PSUM") as psump,
        ):
            ident = constp.tile([128, 128], bf16, tag="ident")
            masks.make_identity(nc, ident[:, :])

            for ck in range(NCHUNK):
                z0 = ck * DZ

                # ---- load base (sy=0) tiles + mask ----
                base = {}
                for f in range(6):
                    t = basep.tile([128, TFREE], f32, tag=f"base{f}")
                    t3 = t[:, :].rearrange("p (z x) -> p z x", z=PLANES)
                    nc.sync.dma_start(
                        out=t3,
                        in_=pad[f, z0 : z0 + PLANES, 2 : 2 + 128, :].transpose(
                            [1, 0, 2]
                        ),
                    )
                    base[f] = t
                mt = constp.tile([128, FDOUT], f32, tag="mask")
                nc.sync.dma_start(
                    out=mt[:, :].rearrange("p (z x) -> p z x", z=DZ),
                    in_=msk[z0 : z0 + DZ, :, :].transpose([1, 0, 2]),
                )

                # ---- PSUM accumulators ----
                wacc = [
                    psump.tile([128, FDOUT], f32, tag=f"w{a}", name=f"wacc{a}")
                    for a in range(3)
                ]

                first = True
                for sy in (0, -1, 1, -2, 2):
                    if sy == 0:
                        cur = base
                    else:
                        cur = {}
                        for f in range(6):
                            t = rollp.tile([128, TFREE], f32, tag=f"roll{f}")
                            t3 = t[:, :].rearrange("p (z x) -> p z x", z=PLANES)
                            nc.sync.dma_start(
                                out=t3,
                                in_=pad[
                                    f,
                                    z0 : z0 + PLANES,
                                    2 - sy : 130 - sy,
                                    :,
                                ].transpose([1, 0, 2]),
                            )
                            cur[f] = t
                    for sz in (-2, -1, 0, 1, 2):
                        for sx in (-2, -1, 0, 1, 2):
                            if sz == 0 and sy == 0 and sx == 0:
                                continue
                            no = W0 - sz * HW_ROW - sx  # neighbor window start
                            cw = [base[f][:, W0 : W0 + WIN] for f in range(6)]
                            nw = [cur[f][:, no : no + WIN] for f in range(6)]

                            # fp32 diffs (cancellation-safe) -> bf16 results
                            dx = longp.tile([128, WIN], bf16, tag="dx")
                            dy = longp.tile([128, WIN], bf16, tag="dy")
                            dzt = longp.tile([128, WIN], bf16, tag="dz")
                            nc.vector.tensor_tensor(dx[:, :], cw[0], nw[0], A.subtract)
                            nc.vector.tensor_tensor(dy[:, :], cw[1], nw[1], A.subtract)
                            nc.vector.tensor_tensor(dzt[:, :], cw[2], nw[2], A.subtract)

                            x2 = scrp.tile([128, WIN], bf16, tag="scrb")
                            y2 = scrp.tile([128, WIN], bf16, tag="scrb")
                            z2 = scrp.tile([128, WIN], bf16, tag="scrb")
                            nc.scalar.square(x2[:, :], dx[:, :])
                            nc.scalar.square(y2[:, :], dy[:, :])
                            nc.scalar.square(z2[:, :], dzt[:, :])
                            s12 = scrp.tile([128, WIN], bf16, tag="scrb")
                            nc.vector.tensor_tensor(s12[:, :], x2[:, :], y2[:, :], A.add)
                            q = longp.tile([128, WIN], bf16, tag="q")
                            nc.vector.tensor_tensor(q[:, :], z2[:, :], s12[:, :], A.add)

                            dvx = scrp.tile([128, WIN], bf16, tag="scrb")
                            dvy = scrp.tile([128, WIN], bf16, tag="scrb")
                            dvz = scrp.tile([128, WIN], bf16, tag="scrb")
                            nc.vector.tensor_tensor(dvx[:, :], cw[3], nw[3], A.subtract)
                            nc.vector.tensor_tensor(dvy[:, :], cw[4], nw[4], A.subtract)
                            nc.vector.tensor_tensor(dvz[:, :], cw[5], nw[5], A.subtract)
                            m1 = scrp.tile([128, WIN], bf16, tag="scrb")
                            m2 = scrp.tile([128, WIN], bf16, tag="scrb")
                            m3 = scrp.tile([128, WIN], bf16, tag="scrb")
                            m4 = scrp.tile([128, WIN], bf16, tag="scrb")
                            dvn = scrp.tile([128, WIN], bf16, tag="scrb")
                            nc.vector.tensor_tensor(m1[:, :], dvx[:, :], dx[:, :], A.mult)
                            nc.vector.tensor_tensor(m2[:, :], dvy[:, :], dy[:, :], A.mult)
                            nc.vector.tensor_tensor(m3[:, :], m1[:, :], m2[:, :], A.add)
                            nc.vector.tensor_tensor(m4[:, :], dvz[:, :], dzt[:, :], A.mult)
                            nc.vector.tensor_tensor(dvn[:, :], m3[:, :], m4[:, :], A.add)
                            # dist = sqrt(q) on ScalarE (fp32); denom = max(dist, EPS);
                            # inv = 1/denom (fast NR reciprocal); invb = bf16 copy
                            dist = scrp.tile([128, WIN], f32, tag="scrf", bufs=4)
                            nc.scalar.sqrt(dist[:, :], q[:, :])
                            den = scrp.tile([128, WIN], f32, tag="scrf", bufs=4)
                            nc.vector.tensor_scalar(
                                den[:, :], dist[:, :], EPS, None, A.max
                            )
                            inv = longp.tile([128, WIN], f32, tag="inv")
                            nc.vector.reciprocal_approx_fast(inv[:, :], den[:, :])
                            invb = longp.tile([128, WIN], bf16, tag="invb")
                            nc.scalar.copy(invb[:, :], inv[:, :])

                            # F = ((dist - 2d) + R_ED*dvn*inv) * inv
                            a1 = scrp.tile([128, WIN], bf16, tag="scrb")
                            nc.scalar.add(a1[:, :], dist[:, :], -TWO_D)
                            b1 = scrp.tile([128, WIN], bf16, tag="scrb")
                            nc.vector.tensor_tensor(
                                b1[:, :], dvn[:, :], invb[:, :], A.mult
                            )
                            c1 = scrp.tile([128, WIN], bf16, tag="scrb")
                            nc.vector.scalar_tensor_tensor(
                                c1[:, :], b1[:, :], R_ED, a1[:, :], A.mult, A.add
                            )
                            F = scrp.tile([128, WIN], bf16, tag="scrb")
                            nc.vector.tensor_tensor(F[:, :], c1[:, :], invb[:, :], A.mult)
                            hit = scrp.tile([128, WIN], bf16, tag="scrb")
                            nc.vector.tensor_scalar(
                                hit[:, :], q[:, :], HIT_THR, None, A.is_lt
                            )
                            Wt = longp.tile([128, WIN], bf16, tag="W")
                            nc.vector.tensor_tensor(Wt[:, :], F[:, :], hit[:, :], A.mult)

                            last = (sy == 2) and (sz == 2) and (sx == 2)
                            for a, dd in enumerate((dx, dy, dzt)):
                                P = scrp.tile([128, 8 * HW_ROW], bf16, tag="scrb")
                                nc.vector.tensor_tensor(
                                    P[:, 0:WIN], Wt[:, :], dd[:, :], A.mult
                                )
                                for g in range(2):  # two 512-wide psum groups
                                    rhs = (
                                        P[:, g * 544 : g * 544 + 544]
                                        .rearrange("p (z x) -> p z x", z=4)[:, :, 0:128]
                                    )
                                    o = (
                                        wacc[a][:, g * 512 : (g + 1) * 512]
                                        .rearrange("p (z x) -> p z x", z=4)
                                    )
                                    nc.tensor.matmul(
                                        o, ident[:, :], rhs, start=first, stop=last
                                    )
                            first = False

                # ---- epilogue: walls + integration ----
                c3 = [
                    base[f][:, :]
                    .rearrange("p (z x) -> p z x", z=PLANES)[:, 2 : 2 + DZ, 4 : 4 + 128]
                    for f in range(6)
                ]
                m3 = mt[:, :].rearrange("p (z x) -> p z x", z=DZ)
                wall_cfg = [
                    # (pos_idx, vel_idx, lo_thr, hi_thr, lo_has_ne0, lo_coeff_base)
                    (0, 3, D, LX - TWO_D, True, D),
                    (1, 4, D, LY - TWO_D, True, D),
                    (2, 5, 3.0 * D, LZ - TWO_D, True, 3.0 * D),
                ]
                for a, (pi, vi, lo_thr, hi_thr, lo_ne0, lo_base) in enumerate(wall_cfg):
                    Xc, Vc = c3[pi], c3[vi]
                    wv = wacc[a][:, :].rearrange("p (z x) -> p z x", z=DZ)
                    il = scrp.tile([128, FDOUT], f32, tag="scr", bufs=6)
                    il3 = il[:, :].rearrange("p (z x) -> p z x", z=DZ)
                    t_a = scrp.tile([128, FDOUT], f32, tag="scr", bufs=6)
                    t_a3 = t_a[:, :].rearrange("p (z x) -> p z x", z=DZ)
                    # il = (pos < lo_thr) & (pos != 0)
                    nc.vector.tensor_scalar(il3, Xc, lo_thr, None, A.is_lt)
                    nc.vector.tensor_scalar(t_a3, Xc, 0.0, None, A.not_equal)
                    nc.vector.tensor_tensor(il3, il3, t_a3, A.mult)
                    ir = scrp.tile([128, FDOUT], f32, tag="scr", bufs=6)
                    ir3 = ir[:, :].rearrange("p (z x) -> p z x", z=DZ)
                    nc.vector.tensor_scalar(ir3, Xc, hi_thr, None, A.is_gt)
                    # wall spring: il*(lo_base - pos) - ir*(pos - hi_thr)
                    u1 = scrp.tile([128, FDOUT], f32, tag="scr", bufs=6)
                    u13 = u1[:, :].rearrange("p (z x) -> p z x", z=DZ)
                    nc.vector.tensor_scalar(u13, Xc, lo_base, -1.0, A.subtract, A.mult)
                    nc.vector.tensor_tensor(u13, u13, il3, A.mult)
                    u2 = scrp.tile([128, FDOUT], f32, tag="scr", bufs=6)
                    u23 = u2[:, :].rearrange("p (z x) -> p z x", z=DZ)
                    nc.vector.tensor_scalar(u23, Xc, hi_thr, None, A.subtract)
                    nc.vector.tensor_tensor(u23, u23, ir3, A.mult)
                    nc.vector.tensor_tensor(u13, u13, u23, A.subtract)
                    # g1 = wall - wacc  (all forces / KN)
                    nc.vector.tensor_tensor(u13, u13, wv, A.subtract)
                    # damp indicator sum
                    nc.vector.tensor_tensor(ir3, il3, ir3, A.add)
                    # g2 = (-C_F*ETA_WALL * vel) * (il+ir)
                    nc.vector.scalar_tensor_tensor(
                        ir3, Vc, -C_F * ETA_WALL, ir3, A.mult, A.mult
                    )
                    # g3 = C_F*KN*g1 + g2
                    nc.vector.scalar_tensor_tensor(
                        u13, u13, C_F * KN, ir3, A.mult, A.add
                    )
                    if a == 2:
                        nc.vector.tensor_scalar(u13, u13, DT * -9.8, None, A.add)
                    # masked
                    nc.vector.tensor_tensor(u13, u13, m3, A.mult)
                    vn = scrp.tile([128, FDOUT], f32, tag="scr", bufs=6)
                    vn3 = vn[:, :].rearrange("p (z x) -> p z x", z=DZ)
                    nc.vector.tensor_tensor(vn3, Vc, u13, A.add)
                    xn = scrp.tile([128, FDOUT], f32, tag="scr", bufs=6)
                    xn3 = xn[:, :].rearrange("p (z x) -> p z x", z=DZ)
                    nc.vector.scalar_tensor_tensor(xn3, vn3, DT, Xc, A.mult, A.add)
                    nc.sync.dma_start(
                        out=out[a, z0 : z0 + DZ, :, :].transpose([1, 0, 2]), in_=xn3
                    )
                    nc.sync.dma_start(
                        out=out[3 + a, z0 : z0 + DZ, :, :].transpose([1, 0, 2]),
                        in_=vn3,
                    )
    nc.compile()
    return nc


_NC = None


def _get_nc():
    global _NC
    if _NC is None:
        _NC = build_nc()
    return _NC


def shard_inputs(x_grid, y_grid, z_grid, vx_grid, vy_grid, vz_grid, mask):
    F = np.stack(
        [
            np.asarray(a, dtype=np.float32).reshape(Z, Y, X)
            for a in (x_grid, y_grid, z_grid, vx_grid, vy_grid, vz_grid)
        ]
    )
    Fp = np.pad(F, ((0, 0), (2, 2), (2, 2), (4, 4)), mode="wrap")
    mk = np.asarray(mask, dtype=np.float32).reshape(Z, Y, X)
    in_maps = []
    for c in range(N_CORES):
        in_maps.append(
            {
                "pad": np.ascontiguousarray(Fp[:, c * ZC : c * ZC + ZC + 4]),
                "msk": np.ascontiguousarray(mk[c * ZC : c * ZC + ZC]),
            }
        )
    return in_maps


def assemble(results):
    full = np.empty((6, 1, 1, Z, Y, X), dtype=np.float32)
    for c in range(N_CORES):
        full[:, 0, 0, c * ZC : (c + 1) * ZC] = results[c]["out"]
    return full


def kernel(**inputs):
    from concourse.bass_utils import run_bass_kernel_spmd

    nc = _get_nc()
    in_maps = shard_inputs(**inputs)
    res = run_bass_kernel_spmd(nc, in_maps, list(range(N_CORES)))
    return assemble(res.results)


# revision 17
# speedup vs baseline: 2.3234x; 1.5256x over previous
"""AI4DEM 5x5x5-stencil DEM single step on 8 TRN2 NeuronCores.

Sharding: Z (dim 2, 256 planes) split 8 ways -> 32 planes/core. Halos (stencil
radius 2, circular per jnp.roll semantics) are materialized host-side by
wrap-padding, so cores are fully independent for the single step.

Device layout per core: partition axis = Y (128). Per z-chunk of DZ=8 output
planes, each field is loaded as [128, (DZ+4)*136] fp32 tiles (z planes
contiguous in the free dim, x padded by 4 per side) in 5 y-rolled variants so
every one of the 124 nonzero (sz,sy,sx) stencil shifts is a pure
free-dim-offset AP. Contact+damping force accumulators (they only ever appear
summed) are accumulated in PSUM via identity matmuls on the TensorEngine.
"""

import os
import sys

sys.path.insert(0, "/opt/trn_rl_repo")

import numpy as np

N_CORES = 8
Z, Y, X = 256, 128, 128
ZC = Z // N_CORES          # z planes per core
DZ = 8                     # z planes per chunk
NCHUNK = int(os.environ.get("DEM_NCHUNK", ZC // DZ))
HW_ROW = 136               # x row width with 4+4 halo
PLANES = DZ + 4            # z planes resident per chunk
TFREE = PLANES * HW_ROW    # free size of an input tile
WIN = 7 * HW_ROW + 128     # flat compute window (plane2 x0 .. plane9 x127)
W0 = 2 * HW_ROW + 4        # window start within a base tile
FDOUT = DZ * X             # 1024, psum accumulator / epilogue free size

CELL = 0.003
D = CELL
TWO_D = 2.0 * D
KN = 10000.0
_REST = 0.5
_ALPHA = -np.log(_REST) / np.pi
_GAMMA = _ALPHA / np.sqrt(_ALPHA**2 + 1.0)
RHO_P = 1592.0
MASS = 4.0 / 3.0 * 3.1415 * CELL**3 * RHO_P
ETA = 2.0 * _GAMMA * np.sqrt(KN * MASS / 2.0)
ETA_WALL = 2.0 * _GAMMA * np.sqrt(KN * MASS)
DT = 0.0001
EPS = 0.0001
LX, LY, LZ = X * CELL, Y * CELL, Z * CELL
C_F = DT / MASS            # velocity update coefficient
R_ED = ETA / KN
HIT_THR = TWO_D * TWO_D          # hit test on q = d2

_SHIFTS = [
    (sz, sy, sx)
    for sy in (0, -1, 1, -2, 2)
    for sz in (-2, -1, 0, 1, 2)
    for sx in (-2, -1, 0, 1, 2)
    if not (sz == 0 and sy == 0 and sx == 0)
]


def build_nc():
    from concourse import bacc, mybir, masks
    from concourse.tile import TileContext

    f32 = mybir.dt.float32
    bf16 = mybir.dt.bfloat16
    A = mybir.AluOpType

    nc = bacc.Bacc()
    # const AP for the ScalarE activation bias (dist - 2d)
    _cb = nc.alloc_sbuf_tensor("const-f32-m2d", [128, 1], f32)
    nc.gpsimd.memset(_cb.ap(), -TWO_D)
    nc.const_aps.aps[(f32, -TWO_D)] = _cb.ap()
    nc.all_engine_barrier()
    pad = nc.declare_dram_parameter(
        "pad", [6, ZC + 4, Y + 4, HW_ROW], f32, isOutput=False
    )
    msk = nc.declare_dram_parameter("msk", [ZC, Y, X], f32, isOutput=False)
    out = nc.declare_dram_parameter("out", [6, ZC, Y, X], f32, isOutput=True)

    with TileContext(nc) as tc:
        with (
            tc.tile_pool(name="const", bufs=1) as constp,
            tc.tile_pool(name="base", bufs=1) as basep,
            tc.tile_pool(name="roll", bufs=1) as rollp,
            tc.tile_pool(name="long", bufs=2) as longp,
            tc.tile_pool(name="scr", bufs=12) as scrp,
            tc.tile_pool(name="psum", bufs=1, space="PSUM") as psump,
        ):
            ident = constp.tile([128, 128], bf16, tag="ident")
            masks.make_identity(nc, ident[:, :])

            for ck in range(NCHUNK):
                z0 = ck * DZ

                # ---- load base (sy=0) tiles + mask ----
                base = {}
                for f in range(6):
                    t = basep.tile([128, TFREE], f32, tag=f"base{f}")
                    t3 = t[:, :].rearrange("p (z x) -> p z x", z=PLANES)
                    nc.sync.dma_start(
                        out=t3,
                        in_=pad[f, z0 : z0 + PLANES, 2 : 2 + 128, :].transpose(
                            [1, 0, 2]
                        ),
                    )
                    base[f] = t
                mt = constp.tile([128, FDOUT], f32, tag="mask")
                nc.sync.dma_start(
                    out=mt[:, :].rearrange("p (z x) -> p z x", z=DZ),
                    in_=msk[z0 : z0 + DZ, :, :].transpose([1, 0, 2]),
                )

                # ---- PSUM accumulators ----
                wacc = [
                    psump.tile([128, FDOUT], f32, tag=f"w{a}", name=f"wacc{a}")
                    for a in range(3)
                ]

                first = True
                for sy in (0, -1, 1, -2, 2):
                    if sy == 0:
                        cur = base
                    else:
                        cur = {}
                        for f in range(6):
                            t = rollp.tile([128, TFREE], f32, tag=f"roll{f}")
                            t3 = t[:, :].rearrange("p (z x) -> p z x", z=PLANES)
                            nc.sync.dma_start(
                                out=t3,
                                in_=pad[
                                    f,
                                    z0 : z0 + PLANES,
                                    2 - sy : 130 - sy,
                                    :,
                                ].transpose([1, 0, 2]),
                            )
                            cur[f] = t
                    for sz in (-2, -1, 0, 1, 2):
                        for sx in (-2, -1, 0, 1, 2):
                            if sz == 0 and sy == 0 and sx == 0:
                                continue
                            no = W0 - sz * HW_ROW - sx  # neighbor window start
                            cw = [base[f][:, W0 : W0 + WIN] for f in range(6)]
                            nw = [cur[f][:, no : no + WIN] for f in range(6)]

                            # fp32 diffs (cancellation-safe) -> bf16 results
                            dx = longp.tile([128, WIN], bf16, tag="dx")
                            dy = longp.tile([128, WIN], bf16, tag="dy")
                            dzt = longp.tile([128, WIN], bf16, tag="dz")
                            nc.vector.tensor_tensor(dx[:, :], cw[0], nw[0], A.subtract)
                            nc.vector.tensor_tensor(dy[:, :], cw[1], nw[1], A.subtract)
                            nc.vector.tensor_tensor(dzt[:, :], cw[2], nw[2], A.subtract)

                            x2 = scrp.tile([128, WIN], bf16, tag="scrb")
                            y2 = scrp.tile([128, WIN], bf16, tag="scrb")
                            z2 = scrp.tile([128, WIN], bf16, tag="scrb")
                            nc.scalar.square(x2[:, :], dx[:, :])
                            nc.scalar.square(y2[:, :], dy[:, :])
                            nc.scalar.square(z2[:, :], dzt[:, :])
                            s12 = scrp.tile([128, WIN], bf16, tag="scrb")
                            nc.vector.tensor_tensor(s12[:, :], x2[:, :], y2[:, :], A.add)
                            q = longp.tile([128, WIN], bf16, tag="q")
                            nc.vector.tensor_tensor(q[:, :], z2[:, :], s12[:, :], A.add)

                            dvx = scrp.tile([128, WIN], bf16, tag="scrb")
                            dvy = scrp.tile([128, WIN], bf16, tag="scrb")
                            dvz = scrp.tile([128, WIN], bf16, tag="scrb")
                            nc.vector.tensor_tensor(dvx[:, :], cw[3], nw[3], A.subtract)
                            nc.vector.tensor_tensor(dvy[:, :], cw[4], nw[4], A.subtract)
                            nc.vector.tensor_tensor(dvz[:, :], cw[5], nw[5], A.subtract)
                            m1 = scrp.tile([128, WIN], bf16, tag="scrb")
                            m2 = scrp.tile([128, WIN], bf16, tag="scrb")
                            m3 = scrp.tile([128, WIN], bf16, tag="scrb")
                            m4 = scrp.tile([128, WIN], bf16, tag="scrb")
                            dvn = scrp.tile([128, WIN], bf16, tag="scrb")
                            nc.vector.tensor_tensor(m1[:, :], dvx[:, :], dx[:, :], A.mult)
                            nc.vector.tensor_tensor(m2[:, :], dvy[:, :], dy[:, :], A.mult)
                            nc.vector.tensor_tensor(m3[:, :], m1[:, :], m2[:, :], A.add)
                            nc.vector.tensor_tensor(m4[:, :], dvz[:, :], dzt[:, :], A.mult)
                            nc.vector.tensor_tensor(dvn[:, :], m3[:, :], m4[:, :], A.add)
                            # qc = max(q, EPS^2); dist = sqrt(qc) (== max(dist,EPS));
                            # inv = 1/dist (fast NR reciprocal); invb = bf16 copy
                            qc = scrp.tile([128, WIN], bf16, tag="scrb")
                            nc.vector.tensor_scalar(
                                qc[:, :], q[:, :], EPS * EPS, None, A.max
                            )
                            dist = scrp.tile([128, WIN], f32, tag="scrf", bufs=4)
                            nc.scalar.sqrt(dist[:, :], qc[:, :])
                            inv = longp.tile([128, WIN], f32, tag="inv")
                            nc.vector.reciprocal_approx_fast(inv[:, :], dist[:, :])
                            invb = longp.tile([128, WIN], bf16, tag="invb")
                            nc.scalar.copy(invb[:, :], inv[:, :])

                            # F = ((dist - 2d) + R_ED*dvn*inv) * inv
                            a1 = scrp.tile([128, WIN], bf16, tag="scrb")
                            nc.scalar.add(a1[:, :], dist[:, :], -TWO_D)
                            b1 = scrp.tile([128, WIN], bf16, tag="scrb")
                            nc.vector.tensor_tensor(
                                b1[:, :], dvn[:, :], invb[:, :], A.mult
                            )
                            c1 = scrp.tile([128, WIN], bf16, tag="scrb")
                            nc.vector.scalar_tensor_tensor(
                                c1[:, :], b1[:, :], R_ED, a1[:, :], A.mult, A.add
                            )
                            F = scrp.tile([128, WIN], bf16, tag="scrb")
                            nc.vector.tensor_tensor(F[:, :], c1[:, :], invb[:, :], A.mult)
                            # hit-mask equivalent: F > 0 off-hit, < 0 on-hit
                            Wt = longp.tile([128, WIN], bf16, tag="W")
                            nc.vector.tensor_scalar(
                                Wt[:, :], F[:, :], 0.0, None, A.min
                            )

                            last = (sy == 2) and (sz == 2) and (sx == 2)
                            for a, dd in enumerate((dx, dy, dzt)):
                                P = scrp.tile([128, 8 * HW_ROW], bf16, tag="scrb")
                                nc.vector.tensor_tensor(
                                    P[:, 0:WIN], Wt[:, :], dd[:, :], A.mult
                                )
                                for g in range(2):  # two 512-wide psum groups
                                    rhs = (
                                        P[:, g * 544 : g * 544 + 544]
                                        .rearrange("p (z x) -> p z x", z=4)[:, :, 0:128]
                                    )
                                    o = (
                                        wacc[a][:, g * 512 : (g + 1) * 512]
                                        .rearrange("p (z x) -> p z x", z=4)
                                    )
                                    nc.tensor.matmul(
                                        o, ident[:, :], rhs, start=first, stop=last
                                    )
                            first = False

                # ---- epilogue: walls + integration ----
                c3 = [
                    base[f][:, :]
                    .rearrange("p (z x) -> p z x", z=PLANES)[:, 2 : 2 + DZ, 4 : 4 + 128]
                    for f in range(6)
                ]
                m3 = mt[:, :].rearrange("p (z x) -> p z x", z=DZ)
                wall_cfg = [
                    # (pos_idx, vel_idx, lo_thr, hi_thr, lo_has_ne0, lo_coeff_base)
                    (0, 3, D, LX - TWO_D, True, D),
                    (1, 4, D, LY - TWO_D, True, D),
                    (2, 5, 3.0 * D, LZ - TWO_D, True, 3.0 * D),
                ]
                for a, (pi, vi, lo_thr, hi_thr, lo_ne0, lo_base) in enumerate(wall_cfg):
                    Xc, Vc = c3[pi], c3[vi]
                    wv = wacc[a][:, :].rearrange("p (z x) -> p z x", z=DZ)
                    il = scrp.tile([128, FDOUT], f32, tag="scr", bufs=6)
                    il3 = il[:, :].rearrange("p (z x) -> p z x", z=DZ)
                    t_a = scrp.tile([128, FDOUT], f32, tag="scr", bufs=6)
                    t_a3 = t_a[:, :].rearrange("p (z x) -> p z x", z=DZ)
                    # il = (pos < lo_thr) & (pos != 0)
                    nc.vector.tensor_scalar(il3, Xc, lo_thr, None, A.is_lt)
                    nc.vector.tensor_scalar(t_a3, Xc, 0.0, None, A.not_equal)
                    nc.vector.tensor_tensor(il3, il3, t_a3, A.mult)
                    ir = scrp.tile([128, FDOUT], f32, tag="scr", bufs=6)
                    ir3 = ir[:, :].rearrange("p (z x) -> p z x", z=DZ)
                    nc.vector.tensor_scalar(ir3, Xc, hi_thr, None, A.is_gt)
                    # wall spring: il*(lo_base - pos) - ir*(pos - hi_thr)
                    u1 = scrp.tile([128, FDOUT], f32, tag="scr", bufs=6)
                    u13 = u1[:, :].rearrange("p (z x) -> p z x", z=DZ)
                    nc.vector.tensor_scalar(u13, Xc, lo_base, -1.0, A.subtract, A.mult)
                    nc.vector.tensor_tensor(u13, u13, il3, A.mult)
                    u2 = scrp.tile([128, FDOUT], f32, tag="scr", bufs=6)
                    u23 = u2[:, :].rearrange("p (z x) -> p z x", z=DZ)
                    nc.vector.tensor_scalar(u23, Xc, hi_thr, None, A.subtract)
                    nc.vector.tensor_tensor(u23, u23, ir3, A.mult)
                    nc.vector.tensor_tensor(u13, u13, u23, A.subtract)
                    # g1 = wall - wacc  (all forces / KN)
                    nc.vector.tensor_tensor(u13, u13, wv, A.subtract)
                    # damp indicator sum
                    nc.vector.tensor_tensor(ir3, il3, ir3, A.add)
                    # g2 = (-C_F*ETA_WALL * vel) * (il+ir)
                    nc.vector.scalar_tensor_tensor(
                        ir3, Vc, -C_F * ETA_WALL, ir3, A.mult, A.mult
                    )
                    # g3 = C_F*KN*g1 + g2
                    nc.vector.scalar_tensor_tensor(
                        u13, u13, C_F * KN, ir3, A.mult, A.add
                    )
                    if a == 2:
                        nc.vector.tensor_scalar(u13, u13, DT * -9.8, None, A.add)
                    # masked
                    nc.vector.tensor_tensor(u13, u13, m3, A.mult)
                    vn = scrp.tile([128, FDOUT], f32, tag="scr", bufs=6)
                    vn3 = vn[:, :].rearrange("p (z x) -> p z x", z=DZ)
                    nc.vector.tensor_tensor(vn3, Vc, u13, A.add)
                    xn = scrp.tile([128, FDOUT], f32, tag="scr", bufs=6)
                    xn3 = xn[:, :].rearrange("p (z x) -> p z x", z=DZ)
                    nc.vector.scalar_tensor_tensor(xn3, vn3, DT, Xc, A.mult, A.add)
                    nc.sync.dma_start(
                        out=out[a, z0 : z0 + DZ, :, :].transpose([1, 0, 2]), in_=xn3
                    )
                    nc.sync.dma_start(
                        out=out[3 + a, z0 : z0 + DZ, :, :].transpose([1, 0, 2]),
                        in_=vn3,
                    )
    nc.compile()
    return nc


_NC = None


def _get_nc():
    global _NC
    if _NC is None:
        _NC = build_nc()
    return _NC


def shard_inputs(x_grid, y_grid, z_grid, vx_grid, vy_grid, vz_grid, mask):
    F = np.stack(
        [
            np.asarray(a, dtype=np.float32).reshape(Z, Y, X)
            for a in (x_grid, y_grid, z_grid, vx_grid, vy_grid, vz_grid)
        ]
    )
    Fp = np.pad(F, ((0, 0), (2, 2), (2, 2), (4, 4)), mode="wrap")
    mk = np.asarray(mask, dtype=np.float32).reshape(Z, Y, X)
    in_maps = []
    for c in range(N_CORES):
        in_maps.append(
            {
                "pad": np.ascontiguousarray(Fp[:, c * ZC : c * ZC + ZC + 4]),
                "msk": np.ascontiguousarray(mk[c * ZC : c * ZC + ZC]),
            }
        )
    return in_maps


def assemble(results):
    full = np.empty((6, 1, 1, Z, Y, X), dtype=np.float32)
    for c in range(N_CORES):
        full[:, 0, 0, c * ZC : (c + 1) * ZC] = results[c]["out"]
    return full


def kernel(**inputs):
    from concourse.bass_utils import run_bass_kernel_spmd

    nc = _get_nc()
    in_maps = shard_inputs(**inputs)
    res = run_bass_kernel_spmd(nc, in_maps, list(range(N_CORES)))
    return assemble(res.results)


# revision 18
# speedup vs baseline: 3.1200x; 1.3429x over previous
"""AI4DEM DEM step, v5: half-shift pairing + cross-chunk carry on 8 TRN2 cores.

Per half-shift s (62 total), phi_s is computed only on the chunk's own z-planes
(x halo'd, +2 z-planes on the core-top chunk). Contributions:
  +phi(c)          -> identity matmul into PSUM planes [2, dz+2)
  -phi(c+s) gather -> shifted-negated-identity matmul, rhs offset by (sz,sx)
  -phi(c') scatter to c'-s below the chunk -> PSUM carry planes [0, 2),
     saved to SBUF and folded into the next-lower chunk's epilogue
Chunks are processed top-down so each chunk's carry is ready for the next.
PSUM: one 8-bank tile [128, 3*(dz+2)*128] f32, explicitly zeroed per chunk.
"""

import os
import sys

sys.path.insert(0, "/opt/trn_rl_repo")

import numpy as np

N_CORES = 8
Z, Y, X = 256, 128, 128
ZC = Z // N_CORES
DZ = 8
HW_ROW = 136                   # x in [-4, 131]
CHUNKS = []
_z = 0
while _z < ZC:
    CHUNKS.append((_z, min(DZ, ZC - _z)))
    _z += DZ
NCHUNK = int(os.environ.get("DEM_NCHUNK", len(CHUNKS)))

CELL = 0.003
D = CELL
TWO_D = 2.0 * D
KN = 10000.0
_REST = 0.5
_ALPHA = -np.log(_REST) / np.pi
_GAMMA = _ALPHA / np.sqrt(_ALPHA**2 + 1.0)
RHO_P = 1592.0
MASS = 4.0 / 3.0 * 3.1415 * CELL**3 * RHO_P
ETA = 2.0 * _GAMMA * np.sqrt(KN * MASS / 2.0)
ETA_WALL = 2.0 * _GAMMA * np.sqrt(KN * MASS)
DT = 0.0001
EPS = 0.0001
LX, LY, LZ = X * CELL, Y * CELL, Z * CELL
C_F = DT / MASS
R_ED = ETA / KN


def half_shifts():
    out = []
    for sy in (0, -1, 1, -2, 2):
        group = []
        for sz in (0, 1, 2):
            for sx in (-2, -1, 0, 1, 2):
                if sz == 0:
                    if sy == 0 and sx <= 0:
                        continue
                    if sy < 0:
                        continue
                group.append((sz, sx))
        out.append((sy, group))
    return out


HALF = half_shifts()
N_HALF = sum(len(g) for _, g in HALF)
assert N_HALF == 62, N_HALF


def bank_groups(s, e):
    """Split f32-column interval [s, e) at 512-col (2KB bank) boundaries."""
    out = []
    while s < e:
        nxt = min(e, (s // 512 + 1) * 512)
        out.append((s, nxt - s))
        s = nxt
    return out


def build_nc():
    from concourse import bacc, mybir, masks
    from concourse.tile import TileContext

    f32 = mybir.dt.float32
    bf16 = mybir.dt.bfloat16
    A = mybir.AluOpType
    ARS = mybir.ActivationFunctionType.Abs_reciprocal_sqrt

    nc = bacc.Bacc()
    TPL = DZ + 4
    TFREE = TPL * HW_ROW

    pad = nc.declare_dram_parameter(
        "pad", [6, ZC + 8, Y + 4, HW_ROW], f32, isOutput=False
    )
    msk = nc.declare_dram_parameter("msk", [ZC, Y, X], f32, isOutput=False)
    out = nc.declare_dram_parameter("out", [6, ZC, Y, X], f32, isOutput=True)

    with TileContext(nc) as tc:
        with (
            tc.tile_pool(name="const", bufs=1) as constp,
            tc.tile_pool(name="base", bufs=1) as basep,
            tc.tile_pool(name="roll", bufs=1) as rollp,
            tc.tile_pool(name="long", bufs=3) as longp,
            tc.tile_pool(name="scr", bufs=14) as scrp,
            tc.tile_pool(name="carry", bufs=2) as carryp,
            tc.tile_pool(name="psum", bufs=1, space="PSUM") as psump,
        ):
            ident = constp.tile([128, 128], bf16, tag="ident")
            masks.make_identity(nc, ident[:, :])
            negs = {}
            for sy in (-2, -1, 0, 1, 2):
                t = constp.tile([128, 128], bf16, tag=f"neg{sy}", name=f"neg{sy}")
                nc.gpsimd.memset(t[:, :], 0.0)
                nc.gpsimd.affine_select(
                    out=t[:, :], in_=t[:, :], compare_op=A.not_equal,
                    fill=-1.0, base=-sy, pattern=[[-1, 128]], channel_multiplier=1,
                )
                if sy != 0:
                    nc.gpsimd.affine_select(
                        out=t[:, :], in_=t[:, :], compare_op=A.not_equal,
                        fill=-1.0, base=-sy + (128 if sy > 0 else -128),
                        pattern=[[-1, 128]], channel_multiplier=1,
                    )
                negs[sy] = t

            carry_prev = None
            for ck in range(NCHUNK - 1, -1, -1):
                z0, dz = CHUNKS[ck]
                ext = 2 if ck == NCHUNK - 1 else 0
                planes = dz + ext + 2          # input: z0-2 .. z0+dz+ext
                wine = (dz + ext - 1) * HW_ROW + 132
                w0 = 2 * HW_ROW + 2            # window: plane z0, x=-2
                fdo = dz * X
                zstr = (dz + 2) * X            # per-axis stride in acc

                base = {}
                for f in range(6):
                    t = basep.tile([128, TFREE], f32, tag=f"base{f}")
                    t3 = t[:, :].rearrange("p (z x) -> p z x", z=TPL)
                    nc.sync.dma_start(
                        out=t3[:, 0:planes, :],
                        in_=pad[f, z0 + 2 : z0 + 2 + planes, 2 : 2 + 128, :]
                        .transpose([1, 0, 2]),
                    )
                    base[f] = t
                mt = constp.tile([128, DZ * X], f32, tag="mask")
                nc.sync.dma_start(
                    out=mt[:, 0:fdo].rearrange("p (z x) -> p z x", z=dz),
                    in_=msk[z0 : z0 + dz, :, :].transpose([1, 0, 2]),
                )

                acc = psump.tile([128, 3 * (DZ + 2) * X], f32, tag="acc")
                nc.vector.memset(acc[:, 0 : 3 * zstr], 0.0)

                for sy, group in HALF:
                    if sy == 0:
                        cur = base
                    else:
                        cur = {}
                        for f in range(6):
                            t = rollp.tile([128, TFREE], f32, tag=f"roll{f}")
                            t3 = t[:, :].rearrange("p (z x) -> p z x", z=TPL)
                            nc.sync.dma_start(
                                out=t3[:, 0:planes, :],
                                in_=pad[
                                    f, z0 + 2 : z0 + 2 + planes, 2 - sy : 130 - sy, :
                                ].transpose([1, 0, 2]),
                            )
                            cur[f] = t
                    for sz, sx in group:
                        no = w0 - sz * HW_ROW - sx
                        cw = [base[f][:, w0 : w0 + wine] for f in range(6)]
                        nw = [cur[f][:, no : no + wine] for f in range(6)]

                        dx = longp.tile([128, wine], bf16, tag="dx")
                        dy = longp.tile([128, wine], bf16, tag="dy")
                        dzt = longp.tile([128, wine], bf16, tag="dz")
                        nc.vector.tensor_tensor(dx[:, :], cw[0], nw[0], A.subtract)
                        nc.vector.tensor_tensor(dy[:, :], cw[1], nw[1], A.subtract)
                        nc.vector.tensor_tensor(dzt[:, :], cw[2], nw[2], A.subtract)

                        x2 = scrp.tile([128, wine], bf16, tag="scrb")
                        y2 = scrp.tile([128, wine], bf16, tag="scrb")
                        z2 = scrp.tile([128, wine], bf16, tag="scrb")
                        nc.scalar.square(x2[:, :], dx[:, :])
                        nc.scalar.square(y2[:, :], dy[:, :])
                        nc.scalar.square(z2[:, :], dzt[:, :])
                        s12 = scrp.tile([128, wine], bf16, tag="scrb")
                        nc.vector.tensor_tensor(s12[:, :], x2[:, :], y2[:, :], A.add)
                        q = scrp.tile([128, wine], bf16, tag="scrb")
                        nc.vector.tensor_tensor(q[:, :], z2[:, :], s12[:, :], A.add)

                        dvx = scrp.tile([128, wine], bf16, tag="scrb")
                        dvy = scrp.tile([128, wine], bf16, tag="scrb")
                        dvz = scrp.tile([128, wine], bf16, tag="scrb")
                        # velocity diffs on GpSimd: overlaps DVE despite the
                        # shared SBUF port pair (measured net win)
                        nc.gpsimd.tensor_tensor(dvx[:, :], cw[3], nw[3], A.subtract)
                        nc.gpsimd.tensor_tensor(dvy[:, :], cw[4], nw[4], A.subtract)
                        nc.gpsimd.tensor_tensor(dvz[:, :], cw[5], nw[5], A.subtract)
                        m1 = scrp.tile([128, wine], bf16, tag="scrb")
                        m2 = scrp.tile([128, wine], bf16, tag="scrb")
                        m3 = scrp.tile([128, wine], bf16, tag="scrb")
                        m4 = scrp.tile([128, wine], bf16, tag="scrb")
                        dvn = scrp.tile([128, wine], bf16, tag="scrb")
                        nc.vector.tensor_tensor(m1[:, :], dvx[:, :], dx[:, :], A.mult)
                        nc.vector.tensor_tensor(m2[:, :], dvy[:, :], dy[:, :], A.mult)
                        nc.vector.tensor_tensor(m3[:, :], m1[:, :], m2[:, :], A.add)
                        nc.vector.tensor_tensor(m4[:, :], dvz[:, :], dzt[:, :], A.mult)
                        nc.vector.tensor_tensor(dvn[:, :], m3[:, :], m4[:, :], A.add)

                        qc = scrp.tile([128, wine], bf16, tag="scrb")
                        nc.vector.tensor_scalar(
                            qc[:, :], q[:, :], EPS * EPS, None, A.max
                        )
                        invb = longp.tile([128, wine], bf16, tag="invb")
                        nc.scalar.activation(invb[:, :], qc[:, :], ARS)

                        E = scrp.tile([128, wine], bf16, tag="scrb")
                        nc.vector.scalar_tensor_tensor(
                            E[:, :], dvn[:, :], R_ED, qc[:, :], A.mult, A.add
                        )
                        t1 = scrp.tile([128, wine], bf16, tag="scrb")
                        nc.vector.tensor_tensor(t1[:, :], E[:, :], invb[:, :], A.mult)
                        F = scrp.tile([128, wine], bf16, tag="scrb")
                        nc.vector.scalar_tensor_tensor(
                            F[:, :], t1[:, :], TWO_D, invb[:, :], A.subtract, A.mult
                        )
                        Wt = longp.tile([128, wine], bf16, tag="W")
                        nc.vector.tensor_scalar(Wt[:, :], F[:, :], 0.0, None, A.min)

                        for a, dd in enumerate((dx, dy, dzt)):
                            P = scrp.tile([128, (DZ + 3) * HW_ROW], bf16, tag="scrb")
                            nc.vector.tensor_tensor(
                                P[:, 0:wine], Wt[:, :], dd[:, :], A.mult
                            )
                            Ab = a * zstr
                            # center: +phi(c), acc planes [2, dz+2)
                            for o, n in bank_groups(Ab + 2 * X, Ab + (dz + 2) * X):
                                cz0 = (o - Ab) // X - 2
                                zp = n // X
                                rhs = P[
                                    :, cz0 * HW_ROW + 2 : (cz0 + zp) * HW_ROW + 2
                                ].rearrange("p (z x) -> p z x", z=zp)[:, :, 0:128]
                                ov = acc[:, o : o + n].rearrange(
                                    "p (z x) -> p z x", z=zp
                                )
                                nc.tensor.matmul(
                                    ov, ident[:, :], rhs,
                                    start=False, stop=False, skip_group_check=True,
                                )
                            # minus-gather: -phi(c+s), c_z in [0, dzg)
                            dzg = dz if ext >= sz else dz - sz
                            for o, n in bank_groups(Ab + 2 * X, Ab + (2 + dzg) * X):
                                cz0 = (o - Ab) // X - 2
                                zp = n // X
                                st = (cz0 + sz) * HW_ROW + sx + 2
                                rhs = P[:, st : st + zp * HW_ROW].rearrange(
                                    "p (z x) -> p z x", z=zp
                                )[:, :, 0:128]
                                ov = acc[:, o : o + n].rearrange(
                                    "p (z x) -> p z x", z=zp
                                )
                                nc.tensor.matmul(
                                    ov, negs[sy][:, :], rhs,
                                    start=False, stop=False, skip_group_check=True,
                                )
                            # minus-carry: -phi(c'), c'_z in [0, sz) -> planes [2-sz, 2)
                            if sz > 0:
                                for o, n in bank_groups(
                                    Ab + (2 - sz) * X, Ab + 2 * X
                                ):
                                    pz0 = (o - Ab) // X
                                    zp = n // X
                                    st = (pz0 - 2 + sz) * HW_ROW + sx + 2
                                    rhs = P[:, st : st + zp * HW_ROW].rearrange(
                                        "p (z x) -> p z x", z=zp
                                    )[:, :, 0:128]
                                    ov = acc[:, o : o + n].rearrange(
                                        "p (z x) -> p z x", z=zp
                                    )
                                    nc.tensor.matmul(
                                        ov, negs[sy][:, :], rhs,
                                        start=False, stop=False,
                                        skip_group_check=True,
                                    )

                # save carry planes for the next-lower chunk
                carry = None
                if ck > 0:
                    carry = carryp.tile([128, 3 * 2 * X], f32, tag="carry")
                    for a in range(3):
                        nc.vector.tensor_copy(
                            carry[:, a * 2 * X : (a + 1) * 2 * X],
                            acc[:, a * zstr : a * zstr + 2 * X],
                        )

                # ---- epilogue ----
                c3 = [
                    base[f][:, :]
                    .rearrange("p (z x) -> p z x", z=TPL)[:, 2 : 2 + dz, 4 : 4 + 128]
                    for f in range(6)
                ]
                m3v = mt[:, 0:fdo].rearrange("p (z x) -> p z x", z=dz)
                wall_cfg = [
                    (0, 3, D, LX - TWO_D, D),
                    (1, 4, D, LY - TWO_D, D),
                    (2, 5, 3.0 * D, LZ - TWO_D, 3.0 * D),
                ]
                for a, (pi, vi, lo_thr, hi_thr, lo_base) in enumerate(wall_cfg):
                    Xc, Vc = c3[pi], c3[vi]
                    wv = acc[:, a * zstr + 2 * X : a * zstr + (2 + dz) * X].rearrange(
                        "p (z x) -> p z x", z=dz
                    )
                    il = scrp.tile([128, DZ * X], f32, tag="scr", bufs=4)
                    il3 = il[:, 0:fdo].rearrange("p (z x) -> p z x", z=dz)
                    t_a = scrp.tile([128, DZ * X], f32, tag="scr", bufs=4)
                    t_a3 = t_a[:, 0:fdo].rearrange("p (z x) -> p z x", z=dz)
                    nc.vector.tensor_scalar(il3, Xc, lo_thr, None, A.is_lt)
                    nc.vector.tensor_scalar(t_a3, Xc, 0.0, None, A.not_equal)
                    nc.vector.tensor_tensor(il3, il3, t_a3, A.mult)
                    ir = scrp.tile([128, DZ * X], f32, tag="scr", bufs=4)
                    ir3 = ir[:, 0:fdo].rearrange("p (z x) -> p z x", z=dz)
                    nc.vector.tensor_scalar(ir3, Xc, hi_thr, None, A.is_gt)
                    u1 = scrp.tile([128, DZ * X], f32, tag="scr", bufs=4)
                    u13 = u1[:, 0:fdo].rearrange("p (z x) -> p z x", z=dz)
                    nc.vector.tensor_scalar(u13, Xc, lo_base, -1.0, A.subtract, A.mult)
                    nc.vector.tensor_tensor(u13, u13, il3, A.mult)
                    u2 = scrp.tile([128, DZ * X], f32, tag="scr", bufs=4)
                    u23 = u2[:, 0:fdo].rearrange("p (z x) -> p z x", z=dz)
                    nc.vector.tensor_scalar(u23, Xc, hi_thr, None, A.subtract)
                    nc.vector.tensor_tensor(u23, u23, ir3, A.mult)
                    nc.vector.tensor_tensor(u13, u13, u23, A.subtract)
                    nc.vector.tensor_tensor(u13, u13, wv, A.subtract)
                    if carry_prev is not None:
                        top = u1[:, (dz - 2) * X : dz * X].rearrange(
                            "p (z x) -> p z x", z=2
                        )
                        cp = carry_prev[:, a * 2 * X : (a + 1) * 2 * X].rearrange(
                            "p (z x) -> p z x", z=2
                        )
                        nc.vector.tensor_tensor(top, top, cp, A.subtract)
                    nc.vector.tensor_tensor(ir3, il3, ir3, A.add)
                    nc.vector.scalar_tensor_tensor(
                        ir3, Vc, -C_F * ETA_WALL, ir3, A.mult, A.mult
                    )
                    nc.vector.scalar_tensor_tensor(
                        u13, u13, C_F * KN, ir3, A.mult, A.add
                    )
                    if a == 2:
                        nc.vector.tensor_scalar(u13, u13, DT * -9.8, None, A.add)
                    nc.vector.tensor_tensor(u13, u13, m3v, A.mult)
                    vn = scrp.tile([128, DZ * X], f32, tag="scr", bufs=4)
                    vn3 = vn[:, 0:fdo].rearrange("p (z x) -> p z x", z=dz)
                    nc.vector.tensor_tensor(vn3, Vc, u13, A.add)
                    xn = scrp.tile([128, DZ * X], f32, tag="scr", bufs=4)
                    xn3 = xn[:, 0:fdo].rearrange("p (z x) -> p z x", z=dz)
                    nc.vector.scalar_tensor_tensor(xn3, vn3, DT, Xc, A.mult, A.add)
                    nc.sync.dma_start(
                        out=out[a, z0 : z0 + dz, :, :].transpose([1, 0, 2]), in_=xn3
                    )
                    nc.sync.dma_start(
                        out=out[3 + a, z0 : z0 + dz, :, :].transpose([1, 0, 2]),
                        in_=vn3,
                    )
                carry_prev = carry
    nc.compile()
    return nc


_NC = None


def _get_nc():
    global _NC
    if _NC is None:
        _NC = build_nc()
    return _NC


def shard_inputs(x_grid, y_grid, z_grid, vx_grid, vy_grid, vz_grid, mask):
    F = np.stack(
        [
            np.asarray(a, dtype=np.float32).reshape(Z, Y, X)
            for a in (x_grid, y_grid, z_grid, vx_grid, vy_grid, vz_grid)
        ]
    )
    Fp = np.pad(F, ((0, 0), (4, 4), (2, 2), (4, 4)), mode="wrap")
    mk = np.asarray(mask, dtype=np.float32).reshape(Z, Y, X)
    in_maps = []
    for c in range(N_CORES):
        in_maps.append(
            {
                "pad": np.ascontiguousarray(Fp[:, c * ZC : c * ZC + ZC + 8]),
                "msk": np.ascontiguousarray(mk[c * ZC : c * ZC + ZC]),
            }
        )
    return in_maps


def assemble(results):
    full = np.empty((6, 1, 1, Z, Y, X), dtype=np.float32)
    for c in range(N_CORES):
        full[:, 0, 0, c * ZC : (c + 1) * ZC] = results[c]["out"]
    return full


def kernel(**inputs):
    from concourse.bass_utils import run_bass_kernel_spmd

    nc = _get_nc()
    in_maps = shard_inputs(**inputs)
    res = run_bass_kernel_spmd(nc, in_maps, list(range(N_CORES)))
    return assemble(res.results)
